# revision 8
# baseline (speedup 1.0000x reference)
"""Trainium2 Bass kernel for multi-head attention.

Problem: B=4, H=16, S=2048, D=128, fp32.
  scores = (q @ k^T) / sqrt(128); probs = softmax(scores, -1); out = probs @ v

Sharding: 64 (b,h) pairs -> 8 cores x 8 pairs. Fully independent per pair.

V5: all-fp16 datapath, stream-pipelined, exp-batched.  The scalar engine
(exp) is the pacing engine: its cost is 0.833ns/elem + ~160ns fixed per
instruction, so score tiles are batched three-per-activation.  Scores are
computed in [t-tile=128, s-chunk=512] units; three consecutive units land
in one [128, 1536] PSUM super-slot (3 banks) and are consumed by a single
exp instruction.  Two super-slots ping-pong (6 banks) + two [128, 512]
PV accumulators (2 banks) fill PSUM exactly.

The device computes, per (pair, s-chunk), the unnormalized PV accumulation
outT[d,s] and the per-key-partition exp sums eacc[t,s] (fp16); the host
finishes softmax: denom[s] = sum_t eacc[t,s], out = outT.T / denom
(flash-attention-style partial results; the division is 0.01% of FLOPs).

Work streams over all (pair, s-chunk, t-tile) units with a lag-2 group
consume: PV matmuls wait on a 2-group-old exp (long complete), so the
scores matmuls never queue behind a blocked PV and the exp engine never
idles.  fp16 matmuls run at 1 row/cycle with LDWEIGHTS fully hidden
(fp32r self-loading adds 128cy/matmul -- avoided); fp16 DVE adds run in
2x perf mode.
"""

import sys

sys.path.insert(0, "/opt/trn_rl_repo")

import numpy as np

B, H, S, D = 4, 16, 2048, 128
N_CORES = 8
BH = B * H                      # 64 pairs
BH_PER_CORE = BH // N_CORES     # 8
T_TILES = S // 128              # 16
SC = 512                        # s-chunk width
N_CHUNKS = S // SC              # 4
GROUP = 3                       # score units per exp instruction
SCALE = float(D) ** -0.5

_cache = {}


def _build_program():
    import concourse.tile as tile
    from concourse import bacc, mybir

    F32 = mybir.dt.float32
    F16 = mybir.dt.float16

    nc = bacc.Bacc("TRN2", target_bir_lowering=False, debug=False)

    qt = nc.dram_tensor("qt", [BH_PER_CORE, D, S], F16, kind="ExternalInput")
    kt = nc.dram_tensor("kt", [BH_PER_CORE, D, S], F16, kind="ExternalInput")
    # v pre-shuffled on host to [p, t, d] so the load is fully contiguous
    v = nc.dram_tensor("v", [BH_PER_CORE, 128, T_TILES * D], F16, kind="ExternalInput")
    # unnormalized PV accumulation, [pair, d, s]
    ot = nc.dram_tensor("ot", [BH_PER_CORE, D, S], F16, kind="ExternalOutput")
    # per-key-partition exp sums, [pair, chunk, t_part, s_chunk]
    dn = nc.dram_tensor(
        "dn", [BH_PER_CORE, N_CHUNKS, 128, SC], F16, kind="ExternalOutput"
    )

    with tile.TileContext(nc) as tc:
        with (
            tc.tile_pool(name="rin", bufs=2) as rin,
            tc.tile_pool(name="exps", bufs=5) as exps,
            tc.tile_pool(name="accp", bufs=3) as accp,
            tc.tile_pool(name="outs", bufs=3) as outs,
            tc.tile_pool(name="psc", bufs=2, space="PSUM") as psc,
            tc.tile_pool(name="pacc", bufs=2, space="PSUM") as pacc,
        ):
            def issue_loads(i):
                q_r = rin.tile([D, S], F16, tag="q_r", name=f"q_{i}")
                k_r = rin.tile([D, S], F16, tag="k_r", name=f"k_{i}")
                v_r = rin.tile([128, T_TILES, D], F16, tag="v_r", name=f"v_{i}")
                nc.sync.dma_start(out=k_r[:, :384], in_=kt[i, :, :384])
                nc.sync.dma_start(out=q_r[:, :512], in_=qt[i, :, :512])
                nc.sync.dma_start(out=k_r[:, 384:1024], in_=kt[i, :, 384:1024])
                nc.sync.dma_start(out=q_r[:, 512:1024], in_=qt[i, :, 512:1024])
                nc.sync.dma_start(
                    out=v_r[:], in_=v[i].rearrange("p (t d) -> p t d", t=T_TILES)
                )
                nc.sync.dma_start(out=q_r[:, 1024:], in_=qt[i, :, 1024:])
                nc.sync.dma_start(out=k_r[:, 1024:], in_=kt[i, :, 1024:])
                return q_r, k_r, v_r

            class ChunkState:
                """Per (pair, s-chunk) accumulators."""

                def __init__(self, i, c, bufs):
                    self.i, self.c = i, c
                    self.q_r, self.k_r, self.v_r = bufs
                    self.oacc = pacc.tile(
                        [128, SC], F32, tag="oacc", name=f"oacc_{i}_{c}"
                    )
                    self.eacc = accp.tile(
                        [128, SC], F16, tag="eacc", name=f"eacc_{i}_{c}"
                    )

            # stream of all score units, grouped GROUP-at-a-time per exp
            stream = [
                (i, c, t)
                for i in range(BH_PER_CORE)
                for c in range(N_CHUNKS)
                for t in range(T_TILES)
            ]
            groups = [stream[p : p + GROUP] for p in range(0, len(stream), GROUP)]

            # warm up the PE p-state ramp with dummy matmuls that have no
            # DMA dependency, so the first real scores run at full clock
            wsrc = rin.tile([128, 128], F16, tag="wsrc", name="wsrc")
            nc.vector.memset(wsrc[:], 0.0)
            wps = psc.tile([128, GROUP * SC], F32, tag="sc", name="warm")
            for j in range(16):
                nc.tensor.matmul(
                    wps[:, (j % 8) * 64 : (j % 8) * 64 + 64],
                    wsrc[:, :128],
                    wsrc[:, :64],
                    start=True,
                    stop=True,
                )

            pair_bufs = {0: issue_loads(0)}
            chunk_states = {}
            # per stream-unit: (ets_tile, column offset) for its exp output
            ets_ref = {}

            def emit_scores_exp(g):
                units = groups[g]
                w = SC * len(units)
                sc_t = psc.tile([128, GROUP * SC], F32, tag="sc", name=f"sc_{g}")
                for j, (i, c, t) in enumerate(units):
                    if (i, c) not in chunk_states:
                        if c == 0 and i + 1 < BH_PER_CORE and (i + 1) not in pair_bufs:
                            pair_bufs[i + 1] = issue_loads(i + 1)
                        chunk_states[(i, c)] = ChunkState(i, c, pair_bufs[i])
                    st = chunk_states[(i, c)]
                    nc.tensor.matmul(
                        sc_t[:, j * SC : (j + 1) * SC],
                        st.k_r[:, t * 128 : (t + 1) * 128],
                        st.q_r[:, c * SC : (c + 1) * SC],
                        start=True,
                        stop=True,
                    )
                et = exps.tile([128, GROUP * SC], F16, tag="et", name=f"et_{g}")
                nc.scalar.activation(
                    et[:, :w],
                    sc_t[:, :w],
                    mybir.ActivationFunctionType.Exp,
                    scale=SCALE,
                )
                for j, u in enumerate(units):
                    ets_ref[u] = (et, j * SC)

            def consume_group(g):
                for i, c, t in groups[g]:
                    st = chunk_states[(i, c)]
                    et, off = ets_ref[(i, c, t)]
                    nc.tensor.matmul(
                        st.oacc[:],
                        st.v_r[:, t, :],
                        et[:, off : off + SC],
                        start=(t == 0),
                        stop=(t == T_TILES - 1),
                    )
                    if t == 1:
                        e0, o0 = ets_ref[(i, c, 0)]
                        nc.vector.tensor_add(
                            st.eacc[:], e0[:, o0 : o0 + SC], et[:, off : off + SC]
                        )
                    elif t > 1:
                        nc.vector.tensor_add(
                            st.eacc[:], st.eacc[:], et[:, off : off + SC]
                        )
                    if t == T_TILES - 1:
                        # PSUM can't be DMA'd; bounce through SBUF as fp16
                        osb = outs.tile(
                            [128, SC], F16, tag="osb", name=f"osb_{i}_{c}"
                        )
                        nc.vector.tensor_copy(osb[:], st.oacc[:])
                        nc.sync.dma_start(
                            out=ot[i, :, c * SC : (c + 1) * SC], in_=osb[:]
                        )
                        nc.sync.dma_start(out=dn[i, c], in_=st.eacc[:])
                        del chunk_states[(i, c)]

            for g in range(len(groups)):
                emit_scores_exp(g)
                # lag-2 consume keeps the scores matmuls off blocked PVs
                if g >= 2:
                    consume_group(g - 2)
            consume_group(len(groups) - 2)
            consume_group(len(groups) - 1)

    nc.finalize()
    return nc


def _get_program():
    if "nc" not in _cache:
        _cache["nc"] = _build_program()
    return _cache["nc"]


def _prepare_in_maps(q4, k4, v4):
    """q4/k4/v4: [BH, S, D] fp32 -> per-core input maps (fp16, T-layout)."""
    in_maps = []
    for core in range(N_CORES):
        sl = slice(core * BH_PER_CORE, (core + 1) * BH_PER_CORE)
        in_maps.append(
            {
                "qt": np.ascontiguousarray(
                    q4[sl].transpose(0, 2, 1).astype(np.float16)
                ),
                "kt": np.ascontiguousarray(
                    k4[sl].transpose(0, 2, 1).astype(np.float16)
                ),
                # [i, t*128+p, d] -> [i, p, t*128+d]
                "v": np.ascontiguousarray(
                    v4[sl]
                    .reshape(BH_PER_CORE, T_TILES, 128, D)
                    .transpose(0, 2, 1, 3)
                    .reshape(BH_PER_CORE, 128, T_TILES * D)
                    .astype(np.float16)
                ),
            }
        )
    return in_maps


def kernel(q: np.ndarray, k: np.ndarray, v: np.ndarray) -> np.ndarray:
    from concourse.bass_utils import run_bass_kernel_spmd

    nc = _get_program()

    q4 = np.ascontiguousarray(q, dtype=np.float32).reshape(BH, S, D)
    k4 = np.ascontiguousarray(k, dtype=np.float32).reshape(BH, S, D)
    v4 = np.ascontiguousarray(v, dtype=np.float32).reshape(BH, S, D)

    in_maps = _prepare_in_maps(q4, k4, v4)

    res = run_bass_kernel_spmd(nc, in_maps, core_ids=list(range(N_CORES)))

    out = np.empty((BH, S, D), dtype=np.float32)
    for core in range(N_CORES):
        otc = res.results[core]["ot"].astype(np.float32)  # [pair, D, S] unnorm
        dnc = res.results[core]["dn"]  # [pair, N_CHUNKS, 128, SC] f16
        # denom[pair, s] = sum over the 128 key partitions, chunks concatenated
        denom = dnc.astype(np.float32).sum(axis=2).reshape(BH_PER_CORE, S)
        out[core * BH_PER_CORE : (core + 1) * BH_PER_CORE] = otc.transpose(
            0, 2, 1
        ) / denom[:, :, None]
    return out.reshape(B, H, S, D)


# revision 14
# speedup vs baseline: 1.0030x; 1.0030x over previous
"""Trainium2 Bass kernel for multi-head attention.

Problem: B=4, H=16, S=2048, D=128, fp32.
  scores = (q @ k^T) / sqrt(128); probs = softmax(scores, -1); out = probs @ v

Sharding: 64 (b,h) pairs -> 8 cores x 8 pairs. Fully independent per pair.

V5: all-fp16 datapath, stream-pipelined, exp-batched.  The scalar engine
(exp) is the pacing engine: its cost is 0.833ns/elem + ~160ns fixed per
instruction, so score tiles are batched three-per-activation.  Scores are
computed in [t-tile=128, s-chunk=512] units; three consecutive units land
in one [128, 1536] PSUM super-slot (3 banks) and are consumed by a single
exp instruction.  Two super-slots ping-pong (6 banks) + two [128, 512]
PV accumulators (2 banks) fill PSUM exactly.

The device computes, per (pair, s-chunk), the unnormalized PV accumulation
outT[d,s] and the per-key-partition exp sums eacc[t,s] (fp16); the host
finishes softmax: denom[s] = sum_t eacc[t,s], out = outT.T / denom
(flash-attention-style partial results; the division is 0.01% of FLOPs).

Work streams over all (pair, s-chunk, t-tile) units with a lag-2 group
consume: PV matmuls wait on a 2-group-old exp (long complete), so the
scores matmuls never queue behind a blocked PV and the exp engine never
idles.  fp16 matmuls run at 1 row/cycle with LDWEIGHTS fully hidden
(fp32r self-loading adds 128cy/matmul -- avoided); fp16 DVE adds run in
2x perf mode.
"""

import sys

sys.path.insert(0, "/opt/trn_rl_repo")

import numpy as np

B, H, S, D = 4, 16, 2048, 128
N_CORES = 8
BH = B * H                      # 64 pairs
BH_PER_CORE = BH // N_CORES     # 8
T_TILES = S // 128              # 16
SC = 512                        # s-chunk width
N_CHUNKS = S // SC              # 4
GROUP = 3                       # score units per exp instruction
SCALE = float(D) ** -0.5

_cache = {}


def _build_program():
    import concourse.tile as tile
    from concourse import bacc, mybir

    F32 = mybir.dt.float32
    F16 = mybir.dt.float16

    nc = bacc.Bacc("TRN2", target_bir_lowering=False, debug=False)

    qt = nc.dram_tensor("qt", [BH_PER_CORE, D, S], F16, kind="ExternalInput")
    kt = nc.dram_tensor("kt", [BH_PER_CORE, D, S], F16, kind="ExternalInput")
    # v pre-shuffled on host to [p, t, d] so the load is fully contiguous
    v = nc.dram_tensor("v", [BH_PER_CORE, 128, T_TILES * D], F16, kind="ExternalInput")
    # unnormalized PV accumulation, [pair, d, s]
    ot = nc.dram_tensor("ot", [BH_PER_CORE, D, S], F16, kind="ExternalOutput")
    # per-key-partition exp sums, [pair, chunk, t_part, s_chunk]
    dn = nc.dram_tensor(
        "dn", [BH_PER_CORE, N_CHUNKS, 128, SC], F16, kind="ExternalOutput"
    )

    with tile.TileContext(nc) as tc:
        with (
            tc.tile_pool(name="rin", bufs=2) as rin,
            tc.tile_pool(name="exps", bufs=5) as exps,
            tc.tile_pool(name="accp", bufs=3) as accp,
            tc.tile_pool(name="outs", bufs=3) as outs,
            tc.tile_pool(name="psc", bufs=2, space="PSUM") as psc,
            tc.tile_pool(name="pacc", bufs=2, space="PSUM") as pacc,
        ):
            def issue_loads(i):
                q_r = rin.tile([D, S], F16, tag="q_r", name=f"q_{i}")
                k_r = rin.tile([D, S], F16, tag="k_r", name=f"k_{i}")
                v_r = rin.tile([128, T_TILES, D], F16, tag="v_r", name=f"v_{i}")
                nc.sync.dma_start(out=k_r[:, :384], in_=kt[i, :, :384])
                nc.sync.dma_start(out=q_r[:, :512], in_=qt[i, :, :512])
                nc.sync.dma_start(out=k_r[:, 384:1024], in_=kt[i, :, 384:1024])
                nc.sync.dma_start(out=q_r[:, 512:1024], in_=qt[i, :, 512:1024])
                nc.sync.dma_start(
                    out=v_r[:], in_=v[i].rearrange("p (t d) -> p t d", t=T_TILES)
                )
                nc.sync.dma_start(out=q_r[:, 1024:], in_=qt[i, :, 1024:])
                nc.sync.dma_start(out=k_r[:, 1024:], in_=kt[i, :, 1024:])
                return q_r, k_r, v_r

            class ChunkState:
                """Per (pair, s-chunk) accumulators."""

                def __init__(self, i, c, bufs):
                    self.i, self.c = i, c
                    self.q_r, self.k_r, self.v_r = bufs
                    self.oacc = pacc.tile(
                        [128, SC], F32, tag="oacc", name=f"oacc_{i}_{c}"
                    )
                    self.eacc = accp.tile(
                        [128, SC], F16, tag="eacc", name=f"eacc_{i}_{c}"
                    )

            # stream of all score units, grouped GROUP-at-a-time per exp
            stream = [
                (i, c, t)
                for i in range(BH_PER_CORE)
                for c in range(N_CHUNKS)
                for t in range(T_TILES)
            ]
            groups = [stream[p : p + GROUP] for p in range(0, len(stream), GROUP)]

            pair_bufs = {0: issue_loads(0)}
            chunk_states = {}
            # per stream-unit: (ets_tile, column offset) for its exp output
            ets_ref = {}

            def emit_scores_exp(g):
                units = groups[g]
                w = SC * len(units)
                sc_t = psc.tile([128, GROUP * SC], F32, tag="sc", name=f"sc_{g}")
                for j, (i, c, t) in enumerate(units):
                    if (i, c) not in chunk_states:
                        if c == 0 and i + 1 < BH_PER_CORE and (i + 1) not in pair_bufs:
                            pair_bufs[i + 1] = issue_loads(i + 1)
                        chunk_states[(i, c)] = ChunkState(i, c, pair_bufs[i])
                    st = chunk_states[(i, c)]
                    nc.tensor.matmul(
                        sc_t[:, j * SC : (j + 1) * SC],
                        st.k_r[:, t * 128 : (t + 1) * 128],
                        st.q_r[:, c * SC : (c + 1) * SC],
                        start=True,
                        stop=True,
                    )
                et = exps.tile([128, GROUP * SC], F16, tag="et", name=f"et_{g}")
                nc.scalar.activation(
                    et[:, :w],
                    sc_t[:, :w],
                    mybir.ActivationFunctionType.Exp,
                    scale=SCALE,
                )
                for j, u in enumerate(units):
                    ets_ref[u] = (et, j * SC)

            def consume_adds(g):
                for i, c, t in groups[g]:
                    st = chunk_states[(i, c)]
                    et, off = ets_ref[(i, c, t)]
                    if t == 1:
                        e0, o0 = ets_ref[(i, c, 0)]
                        nc.vector.tensor_add(
                            st.eacc[:], e0[:, o0 : o0 + SC], et[:, off : off + SC]
                        )
                    elif t > 1:
                        nc.vector.tensor_add(
                            st.eacc[:], st.eacc[:], et[:, off : off + SC]
                        )

            def consume_pv(g):
                for i, c, t in groups[g]:
                    st = chunk_states[(i, c)]
                    et, off = ets_ref[(i, c, t)]
                    nc.tensor.matmul(
                        st.oacc[:],
                        st.v_r[:, t, :],
                        et[:, off : off + SC],
                        start=(t == 0),
                        stop=(t == T_TILES - 1),
                    )
                    if t == T_TILES - 1:
                        # dn first: it only needs the add chain (lag-1),
                        # so its DMA overlaps the cast below
                        nc.sync.dma_start(out=dn[i, c], in_=st.eacc[:])
                        # PSUM can't be DMA'd; bounce through SBUF as fp16
                        osb = outs.tile(
                            [128, SC], F16, tag="osb", name=f"osb_{i}_{c}"
                        )
                        nc.vector.tensor_copy(osb[:], st.oacc[:])
                        nc.sync.dma_start(
                            out=ot[i, :, c * SC : (c + 1) * SC], in_=osb[:]
                        )
                        del chunk_states[(i, c)]

            for g in range(len(groups)):
                emit_scores_exp(g)
                # adds at lag-1 (only gated by the exp), PVs at lag-2 so the
                # scores matmuls never queue behind a blocked PV
                if g >= 1:
                    consume_adds(g - 1)
                if g >= 2:
                    consume_pv(g - 2)
            consume_adds(len(groups) - 1)
            consume_pv(len(groups) - 2)
            consume_pv(len(groups) - 1)

    nc.finalize()
    return nc


def _get_program():
    if "nc" not in _cache:
        _cache["nc"] = _build_program()
    return _cache["nc"]


def _prepare_in_maps(q4, k4, v4):
    """q4/k4/v4: [BH, S, D] fp32 -> per-core input maps (fp16, T-layout)."""
    in_maps = []
    for core in range(N_CORES):
        sl = slice(core * BH_PER_CORE, (core + 1) * BH_PER_CORE)
        in_maps.append(
            {
                "qt": np.ascontiguousarray(
                    q4[sl].transpose(0, 2, 1).astype(np.float16)
                ),
                "kt": np.ascontiguousarray(
                    k4[sl].transpose(0, 2, 1).astype(np.float16)
                ),
                # [i, t*128+p, d] -> [i, p, t*128+d]
                "v": np.ascontiguousarray(
                    v4[sl]
                    .reshape(BH_PER_CORE, T_TILES, 128, D)
                    .transpose(0, 2, 1, 3)
                    .reshape(BH_PER_CORE, 128, T_TILES * D)
                    .astype(np.float16)
                ),
            }
        )
    return in_maps


def kernel(q: np.ndarray, k: np.ndarray, v: np.ndarray) -> np.ndarray:
    from concourse.bass_utils import run_bass_kernel_spmd

    nc = _get_program()

    q4 = np.ascontiguousarray(q, dtype=np.float32).reshape(BH, S, D)
    k4 = np.ascontiguousarray(k, dtype=np.float32).reshape(BH, S, D)
    v4 = np.ascontiguousarray(v, dtype=np.float32).reshape(BH, S, D)

    in_maps = _prepare_in_maps(q4, k4, v4)

    res = run_bass_kernel_spmd(nc, in_maps, core_ids=list(range(N_CORES)))

    out = np.empty((BH, S, D), dtype=np.float32)
    for core in range(N_CORES):
        otc = res.results[core]["ot"].astype(np.float32)  # [pair, D, S] unnorm
        dnc = res.results[core]["dn"]  # [pair, N_CHUNKS, 128, SC] f16
        # denom[pair, s] = sum over the 128 key partitions, chunks concatenated
        denom = dnc.astype(np.float32).sum(axis=2).reshape(BH_PER_CORE, S)
        out[core * BH_PER_CORE : (core + 1) * BH_PER_CORE] = otc.transpose(
            0, 2, 1
        ) / denom[:, :, None]
    return out.reshape(B, H, S, D)
